# revision 20
# baseline (speedup 1.0000x reference)
"""Cross-attention kernel for Trainium2, 8 NeuronCores, data-parallel over batch.

Algebraic restructuring (weights folded on host, in fp64 -- free):
    Wqk = Wq @ Wk.T        [d, d]
    Wkv = Wk @ Wv          [d, f]
so that per batch b (one batch per core):
    qh     = q[b] @ Wqk            [Nq, d]    (q-projection in the y-feature basis)
    scores = qh @ y[b].T / 32      [Nq, Nk]   (== q_proj @ k_proj.T / sqrt(E))
    v      = y[b] @ Wkv            [Nk, F]    (== k_proj @ Wv)
    out    = softmax(scores) @ v
This removes the entire k-projection (256 matmuls/core) from the device.

Device layout: activations are feature-major ([feature_part, token_free]) so
every matmul contracts along the SBUF partition dim with zero on-device
transposes. The host pre-transposes q/y and pre-casts to fp16 (same PE rate
as bf16, 4x less rounding noise, half the DMA bytes).

The scores matmul (the only place where fp8 noise is attenuated enough --
score std is ~0.43, and softmax turns absolute score error into relative
attn error) runs 3/4 of its contraction in fp8e4 with perf_mode=DoubleRow
(2 fp8 weights per PE cell, contraction 256 per matmul, ~2x FLOP rate) and
1/4 in fp16 -- the hybrid ratio sets the accuracy/speed point. qh is
quantized to fp8 (x8 scale, folded into the exp) on the DVE PSUM->SBUF
copy; y arrives fp8 (x8) from the host. Measured end-to-end error 0.0155
vs the 2e-2 gate (full-fp8 scores measured 0.0189: too thin; the value
path stays fp16 since fp8 there puts ~3% noise straight on out).

scoresT [m, n] = (y8 as lhsT).T @ qh8  -> partition = keys m
exp on ScalarE with 1/(sqrt(E)*64) folded into the activation scale
(scores are small, |s| < ~3, so no max-subtraction needed); the fp16
contraction chunks are pre-scaled x64 on the qh copy to match the fp8
chunks' scale.
Softmax denominator: eT chunks are accumulated into esum[m_part, n] on DVE
as each exp lands (so it's ready before the out matmuls drain), then one
1-column ones-matmul per 128-query group reduces over partitions; the out
block is scaled by the reciprocal on DVE.
"""

import os
os.environ.setdefault("NEURON_RT_RESET_CORES", "1")

import numpy as np
import ml_dtypes
from contextlib import ExitStack

import concourse.bass as bass
import concourse.tile as tile
from concourse import bacc, mybir
from concourse.bass_utils import run_bass_kernel_spmd

P = 128
F32 = mybir.dt.float32
F16 = mybir.dt.float16
FP8 = mybir.dt.float8e4

# Problem shapes (hardcoded per contract)
B = 8
NQ = 2048
NK = 2048
D = 1024   # in_q_dim == in_dim (folded: qh lives in the y-feature basis)
F = 1024   # out_dim (v)

USE_FP8_SCORES = True
FP8_PAIRS = 3      # of DC//2=4 256-wide DoubleRow pairs in the scores contraction
QH8_SCALE = 8.0
Y8_SCALE = 8.0


def build_program(nq=NQ, nk=NK, d=D, f=F, nblk=512, fp8_scores=USE_FP8_SCORES,
                  fp8_pairs=FP8_PAIRS):
    """Single-core Bass program (same program runs SPMD on all cores)."""
    nc = bacc.Bacc(trn_type="TRN2")

    DC = d // P            # feature chunks (contraction for qh/scores/v)
    MC = nk // P           # key chunks (contraction for out)
    MB = nk // 512         # 512-wide key blocks for the v phase / yT DMA
    NB = nq // nblk        # query blocks
    NSUB = nblk // P       # 128-query subblocks per block
    FCH = f // 512         # 512-wide chunks of the value dim
    assert nblk <= 512

    qT = nc.dram_tensor("qT", [d, nq], F16, kind="ExternalInput").ap()
    yT = nc.dram_tensor("yT", [d, nk], F16, kind="ExternalInput").ap()
    Wqk = nc.dram_tensor("Wqk", [d, d], F16, kind="ExternalInput").ap()
    Wkv = nc.dram_tensor("Wkv", [d, f], F16, kind="ExternalInput").ap()
    if fp8_scores:
        yT8 = nc.dram_tensor("yT8", [d, nk], FP8, kind="ExternalInput").ap()
        yT8_v = yT8.rearrange("(c p) n -> p c n", p=P)
    out = nc.dram_tensor("out", [nq, f], F32, kind="ExternalOutput").ap()

    qT_v = qT.rearrange("(c p) n -> p c n", p=P)     # [P, DC, nq]
    yT_v = yT.rearrange("(c p) n -> p c n", p=P)     # [P, DC, nk]
    Wqk_v = Wqk.rearrange("(c p) e -> p c e", p=P)   # [P, DC, d]
    Wkv_v = Wkv.rearrange("(c p) f -> p c f", p=P)   # [P, DC, f]
    out_v = out.rearrange("(b p) f -> b p f", p=P)   # [nq//P, P, f]

    with tile.TileContext(nc) as tc, ExitStack() as ctx:
        consts = ctx.enter_context(tc.tile_pool(name="consts", bufs=1))
        y_pool = ctx.enter_context(tc.tile_pool(name="ysb", bufs=1))
        v_pool = ctx.enter_context(tc.tile_pool(name="vproj", bufs=1))
        wqk_pool = ctx.enter_context(tc.tile_pool(name="wqk", bufs=1))
        staging = ctx.enter_context(tc.tile_pool(name="staging", bufs=2))
        psum_a = ctx.enter_context(
            tc.tile_pool(name="psum_a", bufs=3, space="PSUM"))

        ones16 = consts.tile([P, 1], F16)
        nc.vector.memset(ones16, 1.0)
        zbias = consts.tile([P, 1], F32)
        nc.vector.memset(zbias, 0.0)

        # 8 dummy matmuls on const tiles warm the PE clock gate (HAM)
        # while the startup DMA streams, so the first real matmuls run at
        # 2.4GHz instead of the cold 1.2GHz
        warm_w = consts.tile([P, P], F16)
        warm_x = consts.tile([P, 512], F16)
        nc.vector.memset(warm_w, 0.0)
        nc.vector.memset(warm_x, 0.0)
        for _ in range(8):
            wps = psum_a.tile([P, 512], F32, tag="psa", name="wps")
            nc.tensor.matmul(wps, lhsT=warm_w, rhs=warm_x,
                             start=True, stop=True)

        y_sb = y_pool.tile([P, DC, nk], F16)     # full yT, resident
        v_sb = v_pool.tile([P, MC, f], F16)      # [m_part, m_chunk, f]
        wqk_sb = wqk_pool.tile([P, DC, d], F16)
        if fp8_scores:
            y8_sb = y_pool.tile([P, DC, nk], FP8)

        # ---- Phase 1: v = y @ Wkv (transient Wkv weights) ----
        # weights stream on the scalar-engine DMA queue, activations on the
        # sync queue; j-outer + fine interleave so the first matmul's deps
        # land after ~0.5MB of DMA
        with tc.tile_pool(name="wkv", bufs=1) as wkv_pool:
            wkv_sb = wkv_pool.tile([P, DC, f], F16)
            DSP = 2
            for c in range(0, DC, DSP):
                nc.scalar.dma_start(wkv_sb[:, c:c + DSP, 0:512],
                                    Wkv_v[:, c:c + DSP, 0:512])
                nc.sync.dma_start(y_sb[:, c:c + DSP, 0:512],
                                  yT_v[:, c:c + DSP, 0:512])
            for c in range(0, DC, DSP):
                nc.scalar.dma_start(wkv_sb[:, c:c + DSP, 512:1024],
                                    Wkv_v[:, c:c + DSP, 512:1024])

            for mb in range(MB):
                if mb > 0:
                    nc.sync.dma_start(y_sb[:, :, mb * 512:(mb + 1) * 512],
                                      yT_v[:, :, mb * 512:(mb + 1) * 512])
                if mb == 1:
                    nc.scalar.dma_start(wqk_sb, Wqk_v)
                    if fp8_scores:
                        nc.scalar.dma_start(y8_sb, yT8_v)
                # first block runs j-outer so its first 4 matmul groups only
                # need the j=0 half of Wkv (halves the startup DMA wait)
                if mb == 0:
                    rj = [(r, j) for j in range(FCH) for r in range(512 // P)]
                else:
                    rj = [(r, j) for r in range(512 // P) for j in range(FCH)]
                for r, j in rj:
                    mi = mb * (512 // P) + r
                    ps = psum_a.tile([P, 512], F32, tag="psa", name="psa")
                    for di in range(DC):
                        nc.tensor.matmul(
                            ps,
                            lhsT=y_sb[:, di, mi * P:(mi + 1) * P],
                            rhs=wkv_sb[:, di, j * 512:(j + 1) * 512],
                            start=(di == 0), stop=(di == DC - 1))
                    nc.vector.tensor_copy(v_sb[:, mi, j * 512:(j + 1) * 512], ps)

        # ---- Phase 2: attention, blocked over queries ----
        qh_pool = ctx.enter_context(tc.tile_pool(name="qh", bufs=2))
        eT_pool = ctx.enter_context(tc.tile_pool(name="eT", bufs=2))
        esum_pool = ctx.enter_context(tc.tile_pool(name="esum", bufs=2))
        out_pool = ctx.enter_context(tc.tile_pool(name="outsb", bufs=2))
        small = ctx.enter_context(tc.tile_pool(name="small", bufs=8))
        psum_o = ctx.enter_context(
            tc.tile_pool(name="psum_o", bufs=4, space="PSUM"))
        psum_s = ctx.enter_context(
            tc.tile_pool(name="psum_s", bufs=1, space="PSUM"))

        K8 = (fp8_pairs if fp8_scores else 0)   # DoubleRow pairs (256 wide each)
        C8 = 2 * K8                              # 128-chunks covered by fp8
        exp_scale = 1.0 / float(np.sqrt(d))
        if K8 > 0:
            # fp8 part yields scores*QH8_SCALE*Y8_SCALE; fp16 part is scaled
            # to match on the qh copy
            exp_scale /= QH8_SCALE * Y8_SCALE

        for nb in range(NB):
            qt = staging.tile([P, DC, nblk], F16, tag="stage")
            nc.sync.dma_start(qt, qT_v[:, :, nb * nblk:(nb + 1) * nblk])

            # qhT[d2, n_blk]
            qh8 = (qh_pool.tile([P, C8, nblk], FP8, tag="qh8", name="qh8")
                   if K8 > 0 else None)
            qh16 = (qh_pool.tile([P, DC - C8, nblk], F16, tag="qh16",
                                 name="qh16")
                    if C8 < DC else None)
            for ei in range(DC):
                ps = psum_a.tile([P, 512], F32, tag="psa", name="psa")[:, :nblk]
                for di in range(DC):
                    nc.tensor.matmul(
                        ps,
                        lhsT=wqk_sb[:, di, ei * P:(ei + 1) * P],
                        rhs=qt[:, di, :],
                        start=(di == 0), stop=(di == DC - 1))
                if ei < C8:
                    # quantize to fp8 on DVE; scale folded into exp
                    nc.vector.tensor_scalar_mul(qh8[:, ei, :], ps, QH8_SCALE)
                elif K8 > 0:
                    nc.vector.tensor_scalar_mul(
                        qh16[:, ei - C8, :], ps, QH8_SCALE * Y8_SCALE)
                else:
                    nc.vector.tensor_copy(qh16[:, ei, :], ps)

            # eT[m, n_blk] = exp(scoresT / sqrt(E)); esum accumulates on DVE
            eT = eT_pool.tile([P, MC, nblk], F16)
            esum32 = esum_pool.tile([P, nblk], F32, tag="es32")
            esum16 = esum_pool.tile([P, nblk], F16, tag="es16")
            for mi in range(MC):
                ps = psum_a.tile([P, 512], F32, tag="psa", name="psa")[:, :nblk]
                for dp in range(K8):
                    nc.tensor.matmul(
                        ps,
                        lhsT=y8_sb[:, 2 * dp:2 * dp + 2, mi * P:(mi + 1) * P],
                        rhs=qh8[:, 2 * dp:2 * dp + 2, :],
                        start=(dp == 0), stop=(dp == K8 - 1 and C8 == DC),
                        perf_mode=mybir.MatmulPerfMode.DoubleRow)
                for di in range(C8, DC):
                    nc.tensor.matmul(
                        ps,
                        lhsT=y_sb[:, di, mi * P:(mi + 1) * P],
                        rhs=qh16[:, di - C8, :],
                        start=(di == 0), stop=(di == DC - 1))
                nc.scalar.activation(
                    eT[:, mi, :], ps,
                    mybir.ActivationFunctionType.Exp,
                    bias=zbias, scale=exp_scale)
                if mi == 0:
                    nc.vector.tensor_copy(esum32, eT[:, 0, :])
                else:
                    nc.vector.scalar_tensor_tensor(
                        esum32, eT[:, mi, :], 1.0, esum32,
                        op0=mybir.AluOpType.mult, op1=mybir.AluOpType.add)
            nc.vector.tensor_copy(esum16, esum32)

            # out[n, f] = (eT.T @ v) / (eT.T @ 1)
            for ns in range(NSUB):
                last = (nb == NB - 1 and ns == NSUB - 1)
                pos = [psum_o.tile([P, 512], F32, tag="pso", name="pso")
                       for j in range(FCH)]
                if not last:
                    for mi in range(MC):
                        lhsT_e = eT[:, mi, ns * P:(ns + 1) * P]
                        for j in range(FCH):
                            nc.tensor.matmul(
                                pos[j], lhsT=lhsT_e,
                                rhs=v_sb[:, mi, j * 512:(j + 1) * 512],
                                start=(mi == 0), stop=(mi == MC - 1))
                    pss = psum_s.tile([P, 1], F32, tag="pss", name="pss")
                    nc.tensor.matmul(pss, lhsT=esum16[:, ns * P:(ns + 1) * P],
                                     rhs=ones16, start=True, stop=True)
                    rec = small.tile([P, 1], F32)
                    nc.vector.reciprocal(rec, pss)
                    ob = out_pool.tile([P, f], F32)
                    for j in range(FCH):
                        nc.vector.tensor_scalar_mul(
                            ob[:, j * 512:(j + 1) * 512], pos[j], rec)
                        # j=1 goes out on the (idle) gpsimd queue so stores
                        # of the two halves overlap
                        dq = nc.sync if j == 0 else nc.scalar
                        dq.dma_start(
                            out_v[nb * NSUB + ns][:, j * 512:(j + 1) * 512],
                            ob[:, j * 512:(j + 1) * 512])
                else:
                    # final subblock: finish the j=0 half (den/recip/scale/
                    # store) while the j=1 matmuls still run, so only the
                    # j=1 store is exposed in the tail
                    rec = small.tile([P, 1], F32)
                    ob = out_pool.tile([P, f], F32)
                    for j in range(FCH):
                        for mi in range(MC):
                            nc.tensor.matmul(
                                pos[j], lhsT=eT[:, mi, ns * P:(ns + 1) * P],
                                rhs=v_sb[:, mi, j * 512:(j + 1) * 512],
                                start=(mi == 0), stop=(mi == MC - 1))
                        if j == 0:
                            pss = psum_s.tile([P, 1], F32, tag="pss",
                                              name="pss")
                            nc.tensor.matmul(
                                pss, lhsT=esum16[:, ns * P:(ns + 1) * P],
                                rhs=ones16, start=True, stop=True)
                            nc.vector.reciprocal(rec, pss)
                        nc.vector.tensor_scalar_mul(
                            ob[:, j * 512:(j + 1) * 512], pos[j], rec)
                        dq = nc.sync if j == 0 else nc.scalar
                        dq.dma_start(
                            out_v[nb * NSUB + ns][:, j * 512:(j + 1) * 512],
                            ob[:, j * 512:(j + 1) * 512])

    nc.compile()
    return nc


_CACHE = {}


def make_in_maps(q, y, Wq, Wk, Wv):
    """Host-side prep: fold weights (fp64), transpose + cast."""
    Wqk = (np.asarray(Wq, np.float64) @ np.asarray(Wk, np.float64).T)
    Wkv = (np.asarray(Wk, np.float64) @ np.asarray(Wv, np.float64))
    Wqk16 = np.ascontiguousarray(Wqk, np.float16)
    Wkv16 = np.ascontiguousarray(Wkv, np.float16)
    q = np.asarray(q)
    y = np.asarray(y)
    in_maps = []
    for b in range(B):
        yTb = np.ascontiguousarray(y[b].T, np.float16)
        m = {
            "qT": np.ascontiguousarray(q[b].T, np.float16),
            "yT": yTb,
            "Wqk": Wqk16, "Wkv": Wkv16,
        }
        if USE_FP8_SCORES:
            m["yT8"] = np.clip(y[b].T * Y8_SCALE, -240,
                               240).astype(ml_dtypes.float8_e4m3)
        in_maps.append(m)
    return in_maps


def kernel(q, y, Wq, Wk, Wv):
    if "nc" not in _CACHE:
        _CACHE["nc"] = build_program()
    nc = _CACHE["nc"]
    in_maps = make_in_maps(q, y, Wq, Wk, Wv)
    res = run_bass_kernel_spmd(nc, in_maps, core_ids=list(range(B)))
    return np.stack([res.results[b]["out"] for b in range(B)], axis=0)


# revision 21
# speedup vs baseline: 1.1933x; 1.1933x over previous
"""Cross-attention kernel for Trainium2, 8 NeuronCores, data-parallel over batch.

Algebraic restructuring (weights folded on host, in fp64 -- free):
    Wqk = Wq @ Wk.T        [d, d]
    Wkv = Wk @ Wv          [d, f]
so that per batch b (one batch per core):
    qh     = q[b] @ Wqk            [Nq, d]    (q-projection in the y-feature basis)
    scores = qh @ y[b].T / 32      [Nq, Nk]   (== q_proj @ k_proj.T / sqrt(E))
    v      = y[b] @ Wkv            [Nk, F]    (== k_proj @ Wv)
    out    = softmax(scores) @ v
This removes the entire k-projection (256 matmuls/core) from the device.

Device layout: activations are feature-major ([feature_part, token_free]) so
every matmul contracts along the SBUF partition dim with zero on-device
transposes. The host pre-transposes q/y and pre-casts to fp16 (same PE rate
as bf16, 4x less rounding noise, half the DMA bytes).

The scores matmul (the only place where fp8 noise is attenuated enough --
score std is ~0.43, and softmax turns absolute score error into relative
attn error) runs 3/4 of its contraction in fp8e4 with perf_mode=DoubleRow
(2 fp8 weights per PE cell, contraction 256 per matmul, ~2x FLOP rate) and
1/4 in fp16 -- the hybrid ratio sets the accuracy/speed point. qh is
quantized to fp8 (x8 scale, folded into the exp) on the DVE PSUM->SBUF
copy; y arrives fp8 (x8) from the host. Measured end-to-end error 0.0155
vs the 2e-2 gate (full-fp8 scores measured 0.0189: too thin; the value
path stays fp16 since fp8 there puts ~3% noise straight on out).

scoresT [m, n] = (y8 as lhsT).T @ qh8  -> partition = keys m
exp on ScalarE with 1/(sqrt(E)*64) folded into the activation scale
(scores are small, |s| < ~3, so no max-subtraction needed); the fp16
contraction chunks are pre-scaled x64 on the qh copy to match the fp8
chunks' scale.
Softmax denominator: eT chunks are accumulated into esum[m_part, n] on DVE
as each exp lands (so it's ready before the out matmuls drain), then one
1-column ones-matmul per 128-query group reduces over partitions; the out
block is scaled by the reciprocal on DVE.
"""

import os
os.environ.setdefault("NEURON_RT_RESET_CORES", "1")

import numpy as np
import ml_dtypes
from contextlib import ExitStack

import concourse.bass as bass
import concourse.tile as tile
from concourse import bacc, mybir
from concourse.bass_utils import run_bass_kernel_spmd

P = 128
F32 = mybir.dt.float32
F16 = mybir.dt.float16
FP8 = mybir.dt.float8e4

# Problem shapes (hardcoded per contract)
B = 8
NQ = 2048
NK = 2048
D = 1024   # in_q_dim == in_dim (folded: qh lives in the y-feature basis)
F = 1024   # out_dim (v)

USE_FP8_SCORES = True
FP8_PAIRS = 3      # of DC//2=4 256-wide DoubleRow pairs in the scores contraction
QH8_SCALE = 8.0
Y8_SCALE = 8.0


def build_program(nq=NQ, nk=NK, d=D, f=F, nblk=512, fp8_scores=USE_FP8_SCORES,
                  fp8_pairs=FP8_PAIRS):
    """Single-core Bass program (same program runs SPMD on all cores)."""
    nc = bacc.Bacc(trn_type="TRN2")

    DC = d // P            # feature chunks (contraction for qh/scores/v)
    MC = nk // P           # key chunks (contraction for out)
    MB = nk // 512         # 512-wide key blocks for the v phase / yT DMA
    NB = nq // nblk        # query blocks
    NSUB = nblk // P       # 128-query subblocks per block
    FCH = f // 512         # 512-wide chunks of the value dim
    assert nblk <= 512

    qT = nc.dram_tensor("qT", [d, nq], F16, kind="ExternalInput").ap()
    yT = nc.dram_tensor("yT", [d, nk], F16, kind="ExternalInput").ap()
    Wqk = nc.dram_tensor("Wqk", [d, d], F16, kind="ExternalInput").ap()
    Wkv = nc.dram_tensor("Wkv", [d, f], F16, kind="ExternalInput").ap()
    if fp8_scores:
        yT8 = nc.dram_tensor("yT8", [d, nk], FP8, kind="ExternalInput").ap()
        yT8_v = yT8.rearrange("(c p) n -> p c n", p=P)
    out = nc.dram_tensor("out", [nq, f], F32, kind="ExternalOutput").ap()

    qT_v = qT.rearrange("(c p) n -> p c n", p=P)     # [P, DC, nq]
    yT_v = yT.rearrange("(c p) n -> p c n", p=P)     # [P, DC, nk]
    Wqk_v = Wqk.rearrange("(c p) e -> p c e", p=P)   # [P, DC, d]
    Wkv_v = Wkv.rearrange("(c p) f -> p c f", p=P)   # [P, DC, f]
    out_v = out.rearrange("(b p) f -> b p f", p=P)   # [nq//P, P, f]

    with tile.TileContext(nc) as tc, ExitStack() as ctx:
        consts = ctx.enter_context(tc.tile_pool(name="consts", bufs=1))
        y_pool = ctx.enter_context(tc.tile_pool(name="ysb", bufs=1))
        v_pool = ctx.enter_context(tc.tile_pool(name="vproj", bufs=1))
        wqk_pool = ctx.enter_context(tc.tile_pool(name="wqk", bufs=1))
        staging = ctx.enter_context(tc.tile_pool(name="staging", bufs=2))
        psum_a = ctx.enter_context(
            tc.tile_pool(name="psum_a", bufs=3, space="PSUM"))

        ones16 = consts.tile([P, 1], F16)
        nc.vector.memset(ones16, 1.0)
        zbias = consts.tile([P, 1], F32)
        nc.vector.memset(zbias, 0.0)

        y_sb = y_pool.tile([P, DC, nk], F16)     # full yT, resident
        v_sb = v_pool.tile([P, MC, f], F16)      # [m_part, m_chunk, f]
        wqk_sb = wqk_pool.tile([P, DC, d], F16)
        if fp8_scores:
            y8_sb = y_pool.tile([P, DC, nk], FP8)

        # ---- Phase 1: v = y @ Wkv (transient Wkv weights) ----
        # weights stream on the scalar-engine DMA queue, activations on the
        # sync queue; j-outer + fine interleave so the first matmul's deps
        # land after ~0.5MB of DMA
        with tc.tile_pool(name="wkv", bufs=1) as wkv_pool:
            wkv_sb = wkv_pool.tile([P, DC, f], F16)
            DSP = 2
            for c in range(0, DC, DSP):
                nc.scalar.dma_start(wkv_sb[:, c:c + DSP, 0:512],
                                    Wkv_v[:, c:c + DSP, 0:512])
                nc.sync.dma_start(y_sb[:, c:c + DSP, 0:512],
                                  yT_v[:, c:c + DSP, 0:512])
            for c in range(0, DC, DSP):
                nc.scalar.dma_start(wkv_sb[:, c:c + DSP, 512:1024],
                                    Wkv_v[:, c:c + DSP, 512:1024])

            for mb in range(MB):
                if mb > 0:
                    nc.sync.dma_start(y_sb[:, :, mb * 512:(mb + 1) * 512],
                                      yT_v[:, :, mb * 512:(mb + 1) * 512])
                if mb == 1:
                    nc.scalar.dma_start(wqk_sb, Wqk_v)
                    if fp8_scores:
                        nc.scalar.dma_start(y8_sb, yT8_v)
                # first block runs j-outer so its first 4 matmul groups only
                # need the j=0 half of Wkv (halves the startup DMA wait)
                if mb == 0:
                    rj = [(r, j) for j in range(FCH) for r in range(512 // P)]
                else:
                    rj = [(r, j) for r in range(512 // P) for j in range(FCH)]
                for r, j in rj:
                    mi = mb * (512 // P) + r
                    ps = psum_a.tile([P, 512], F32, tag="psa", name="psa")
                    for di in range(DC):
                        nc.tensor.matmul(
                            ps,
                            lhsT=y_sb[:, di, mi * P:(mi + 1) * P],
                            rhs=wkv_sb[:, di, j * 512:(j + 1) * 512],
                            start=(di == 0), stop=(di == DC - 1))
                    nc.vector.tensor_copy(v_sb[:, mi, j * 512:(j + 1) * 512], ps)

        # ---- Phase 2: attention, blocked over queries ----
        qh_pool = ctx.enter_context(tc.tile_pool(name="qh", bufs=2))
        eT_pool = ctx.enter_context(tc.tile_pool(name="eT", bufs=2))
        esum_pool = ctx.enter_context(tc.tile_pool(name="esum", bufs=2))
        out_pool = ctx.enter_context(tc.tile_pool(name="outsb", bufs=2))
        small = ctx.enter_context(tc.tile_pool(name="small", bufs=8))
        psum_o = ctx.enter_context(
            tc.tile_pool(name="psum_o", bufs=4, space="PSUM"))
        psum_s = ctx.enter_context(
            tc.tile_pool(name="psum_s", bufs=1, space="PSUM"))

        K8 = (fp8_pairs if fp8_scores else 0)   # DoubleRow pairs (256 wide each)
        C8 = 2 * K8                              # 128-chunks covered by fp8
        exp_scale = 1.0 / float(np.sqrt(d))
        if K8 > 0:
            # fp8 part yields scores*QH8_SCALE*Y8_SCALE; fp16 part is scaled
            # to match on the qh copy
            exp_scale /= QH8_SCALE * Y8_SCALE

        for nb in range(NB):
            qt = staging.tile([P, DC, nblk], F16, tag="stage")
            nc.sync.dma_start(qt, qT_v[:, :, nb * nblk:(nb + 1) * nblk])

            # qhT[d2, n_blk]
            qh8 = (qh_pool.tile([P, C8, nblk], FP8, tag="qh8", name="qh8")
                   if K8 > 0 else None)
            qh16 = (qh_pool.tile([P, DC - C8, nblk], F16, tag="qh16",
                                 name="qh16")
                    if C8 < DC else None)
            for ei in range(DC):
                ps = psum_a.tile([P, 512], F32, tag="psa", name="psa")[:, :nblk]
                for di in range(DC):
                    nc.tensor.matmul(
                        ps,
                        lhsT=wqk_sb[:, di, ei * P:(ei + 1) * P],
                        rhs=qt[:, di, :],
                        start=(di == 0), stop=(di == DC - 1))
                if ei < C8:
                    # quantize to fp8 on DVE; scale folded into exp
                    nc.vector.tensor_scalar_mul(qh8[:, ei, :], ps, QH8_SCALE)
                elif K8 > 0:
                    nc.vector.tensor_scalar_mul(
                        qh16[:, ei - C8, :], ps, QH8_SCALE * Y8_SCALE)
                else:
                    nc.vector.tensor_copy(qh16[:, ei, :], ps)

            # eT[m, n_blk] = exp(scoresT / sqrt(E)); esum accumulates on DVE
            eT = eT_pool.tile([P, MC, nblk], F16)
            esum32 = esum_pool.tile([P, nblk], F32, tag="es32")
            esum16 = esum_pool.tile([P, nblk], F16, tag="es16")
            for mi in range(MC):
                ps = psum_a.tile([P, 512], F32, tag="psa", name="psa")[:, :nblk]
                for dp in range(K8):
                    nc.tensor.matmul(
                        ps,
                        lhsT=y8_sb[:, 2 * dp:2 * dp + 2, mi * P:(mi + 1) * P],
                        rhs=qh8[:, 2 * dp:2 * dp + 2, :],
                        start=(dp == 0), stop=(dp == K8 - 1 and C8 == DC),
                        perf_mode=mybir.MatmulPerfMode.DoubleRow)
                for di in range(C8, DC):
                    nc.tensor.matmul(
                        ps,
                        lhsT=y_sb[:, di, mi * P:(mi + 1) * P],
                        rhs=qh16[:, di - C8, :],
                        start=(di == 0), stop=(di == DC - 1))
                nc.scalar.activation(
                    eT[:, mi, :], ps,
                    mybir.ActivationFunctionType.Exp,
                    bias=zbias, scale=exp_scale)
                if mi == 0:
                    nc.vector.tensor_copy(esum32, eT[:, 0, :])
                else:
                    nc.vector.scalar_tensor_tensor(
                        esum32, eT[:, mi, :], 1.0, esum32,
                        op0=mybir.AluOpType.mult, op1=mybir.AluOpType.add)
            nc.vector.tensor_copy(esum16, esum32)

            # out[n, f] = (eT.T @ v) / (eT.T @ 1)
            for ns in range(NSUB):
                last = (nb == NB - 1 and ns == NSUB - 1)
                pos = [psum_o.tile([P, 512], F32, tag="pso", name="pso")
                       for j in range(FCH)]
                if not last:
                    for mi in range(MC):
                        lhsT_e = eT[:, mi, ns * P:(ns + 1) * P]
                        for j in range(FCH):
                            nc.tensor.matmul(
                                pos[j], lhsT=lhsT_e,
                                rhs=v_sb[:, mi, j * 512:(j + 1) * 512],
                                start=(mi == 0), stop=(mi == MC - 1))
                    pss = psum_s.tile([P, 1], F32, tag="pss", name="pss")
                    nc.tensor.matmul(pss, lhsT=esum16[:, ns * P:(ns + 1) * P],
                                     rhs=ones16, start=True, stop=True)
                    rec = small.tile([P, 1], F32)
                    nc.vector.reciprocal(rec, pss)
                    ob = out_pool.tile([P, f], F32)
                    for j in range(FCH):
                        nc.vector.tensor_scalar_mul(
                            ob[:, j * 512:(j + 1) * 512], pos[j], rec)
                        # j=1 goes out on the (idle) gpsimd queue so stores
                        # of the two halves overlap
                        dq = nc.sync if j == 0 else nc.gpsimd
                        dq.dma_start(
                            out_v[nb * NSUB + ns][:, j * 512:(j + 1) * 512],
                            ob[:, j * 512:(j + 1) * 512])
                else:
                    # final subblock: finish the j=0 half (den/recip/scale/
                    # store) while the j=1 matmuls still run, so only the
                    # j=1 store is exposed in the tail
                    rec = small.tile([P, 1], F32)
                    ob = out_pool.tile([P, f], F32)
                    for j in range(FCH):
                        for mi in range(MC):
                            nc.tensor.matmul(
                                pos[j], lhsT=eT[:, mi, ns * P:(ns + 1) * P],
                                rhs=v_sb[:, mi, j * 512:(j + 1) * 512],
                                start=(mi == 0), stop=(mi == MC - 1))
                        if j == 0:
                            pss = psum_s.tile([P, 1], F32, tag="pss",
                                              name="pss")
                            nc.tensor.matmul(
                                pss, lhsT=esum16[:, ns * P:(ns + 1) * P],
                                rhs=ones16, start=True, stop=True)
                            nc.vector.reciprocal(rec, pss)
                        nc.vector.tensor_scalar_mul(
                            ob[:, j * 512:(j + 1) * 512], pos[j], rec)
                        dq = nc.sync if j == 0 else nc.gpsimd
                        dq.dma_start(
                            out_v[nb * NSUB + ns][:, j * 512:(j + 1) * 512],
                            ob[:, j * 512:(j + 1) * 512])

    nc.compile()
    return nc


_CACHE = {}


def make_in_maps(q, y, Wq, Wk, Wv):
    """Host-side prep: fold weights (fp64), transpose + cast."""
    Wqk = (np.asarray(Wq, np.float64) @ np.asarray(Wk, np.float64).T)
    Wkv = (np.asarray(Wk, np.float64) @ np.asarray(Wv, np.float64))
    Wqk16 = np.ascontiguousarray(Wqk, np.float16)
    Wkv16 = np.ascontiguousarray(Wkv, np.float16)
    q = np.asarray(q)
    y = np.asarray(y)
    in_maps = []
    for b in range(B):
        yTb = np.ascontiguousarray(y[b].T, np.float16)
        m = {
            "qT": np.ascontiguousarray(q[b].T, np.float16),
            "yT": yTb,
            "Wqk": Wqk16, "Wkv": Wkv16,
        }
        if USE_FP8_SCORES:
            m["yT8"] = np.clip(y[b].T * Y8_SCALE, -240,
                               240).astype(ml_dtypes.float8_e4m3)
        in_maps.append(m)
    return in_maps


def kernel(q, y, Wq, Wk, Wv):
    if "nc" not in _CACHE:
        _CACHE["nc"] = build_program()
    nc = _CACHE["nc"]
    in_maps = make_in_maps(q, y, Wq, Wk, Wv)
    res = run_bass_kernel_spmd(nc, in_maps, core_ids=list(range(B)))
    return np.stack([res.results[b]["out"] for b in range(B)], axis=0)
